# revision 2
# baseline (speedup 1.0000x reference)
"""TRN2 Bass kernel: fused LSTM cell (nn_CustomLSTMCell), 8-core tensor-parallel.

Strategy
--------
gates = x @ W_ih.T + b_ih + h_prev @ W_hh.T + b_hh  is computed as ONE GEMM
with contraction K = I + H = 4096 over xh = [x | h_prev] and W = [W_ih | W_hh].

The 4H gate dimension is tensor-parallel sharded across the 8 cores: core c
owns h-columns [c*256, (c+1)*256) of every gate (i, f, g, o).  Each core
computes gatesT [1024, 2048] = Wc @ xh.T with gate rows on partitions, so the
per-gate bias is a native per-partition scalar in scalar.activation, which
also applies sigmoid/tanh while evicting PSUM -> SBUF.  The LSTM cell update
(new_C = f*C + i*g, new_h = o*tanh(new_C)) runs on the vector engine, fully
overlapped with the tensor engine.  No collectives: output slices are
disjoint and gathered on the host.

Matmul operands are cast to fp16 on the host (halves DMA traffic, 4x PE rate
vs fp32, ~8x more mantissa than bf16); accumulation stays fp32 in PSUM and
the epilogue is fp32.

DMA plan: all HBM traffic uses few, large transfers with multi-KB contiguous
per-partition lines (host pre-arranges every tensor partition-major).  The
w halves stream on the Sync HWDGE queue in 1MB blocks, xh generations on the
Scalar HWDGE queue, later xh generations on the GpSimd queue, and each
group's six outputs are packed into one [128, 3072] tile stored with a
single DMA.  Small per-chunk DMAs (the old scheme) were dispatch-bound at
~116GB/s/queue and starved the PE for ~22us at the start of the stream.
"""

import numpy as np

B = 2048           # batch
I_DIM = 2048       # input features
H = 2048           # hidden
NCORES = 8
S = H // NCORES    # 256: per-core h-slice (per gate)
M_PER_CORE = 4 * S # 1024 gate rows per core
K = I_DIM + H      # 4096 fused contraction dim
P = 128
KC = K // P        # 32 contraction chunks
NT = B // 512      # 4 batch tiles of 512
HB = S // P        # 2 h-blocks of 128 per core
NQ = 6             # packed outputs per group: f,i,g,cn,o,h

_BF16 = np.float16

_CACHE = {}

# w/xh k-chunk DMA blocks: (start, stop) chunk ranges.  First block small so
# the PE's first real matmul starts early; later blocks big for DMA
# efficiency (>=1MB, 4-12KB contiguous lines).
_WBLOCKS = [(0, 4), (4, 12), (12, 22), (22, 32)]


def _build_program():
    from contextlib import ExitStack

    import concourse.mybir as mybir
    import concourse.tile as tile
    from concourse import bacc

    f32 = mybir.dt.float32
    bf16 = mybir.dt.float16
    AF = mybir.ActivationFunctionType

    nc = bacc.Bacc("TRN2", target_bir_lowering=False, debug=False)

    # Host layouts are partition-major so every DMA moves multi-KB
    # contiguous lines per partition.
    w_lo_d = nc.dram_tensor("w_lo", [P * KC, 4 * P], bf16, kind="ExternalInput").ap()
    w_hi_d = nc.dram_tensor("w_hi", [P * KC, 4 * P], bf16, kind="ExternalInput").ap()
    xh_d = [
        nc.dram_tensor(f"xh{n}", [P * KC, 512], bf16, kind="ExternalInput").ap()
        for n in range(NT)
    ]
    bias_d = nc.dram_tensor("bias", [P, 4 * HB], f32, kind="ExternalInput").ap()
    c_d = nc.dram_tensor("c_t", [P * HB, B], f32, kind="ExternalInput").ap()
    out_d = nc.dram_tensor(
        "out", [NT * HB * P, NQ * 512], f32, kind="ExternalOutput"
    ).ap()

    w_r = [
        w_lo_d.rearrange("(p a) m -> p a m", p=P),   # [128, 32, 512]
        w_hi_d.rearrange("(p a) m -> p a m", p=P),
    ]
    xh_r = [t.rearrange("(p a) m -> p a m", p=P) for t in xh_d]
    c_r = c_d.rearrange("(p h) m -> p h m", p=P)     # [128, 2, 2048]
    out_r = out_d.rearrange("(g p) m -> g p m", g=NT * HB)  # [8, 128, 3072]

    # gate order within a group: i, f, g, o (torch chunk order); packed
    # output column order: f, i, g, cn, o, h (early finishers first so the
    # final group's tail store can fire before o/h are ready).
    ACT_FN = [AF.Sigmoid, AF.Sigmoid, AF.Tanh, AF.Sigmoid]
    QCOL = {"i": 1, "f": 0, "g": 2, "cn": 3, "o": 4, "h": 5}

    with tile.TileContext(nc) as tc, ExitStack() as ctx:
        w_pool = ctx.enter_context(tc.tile_pool(name="w", bufs=1))
        xh_pool = ctx.enter_context(tc.tile_pool(name="xh", bufs=2))
        c_pool = ctx.enter_context(tc.tile_pool(name="c", bufs=1))
        b_pool = ctx.enter_context(tc.tile_pool(name="b", bufs=1))
        psum_pool = ctx.enter_context(tc.tile_pool(name="ps", bufs=2, space="PSUM"))
        out_pool = ctx.enter_context(tc.tile_pool(name="out", bufs=2))
        tmp_pool = ctx.enter_context(tc.tile_pool(name="tmp", bufs=2))

        # Load bias first: tiny, and the first epilogue needs it.
        bias_all = b_pool.tile([P, 4 * HB], f32)
        nc.sync.dma_start(bias_all[:], bias_d[:, :])

        # A few matmuls on dummy data bridge the framework-preamble gap so
        # HAM warm-up overlaps the first block DMAs.  Never read.
        dummy = b_pool.tile([P, 512], bf16)
        nc.vector.memset(dummy[:], 0.0)
        warm_ps = psum_pool.tile([P, 512], f32, name="ps0")
        NWARM = 10
        for i in range(NWARM):
            nc.tensor.matmul(
                warm_ps[:], dummy[:, 0:P], dummy[:],
                start=(i == 0), stop=(i == NWARM - 1),
            )

        # Input streams.  Sync: w-lo, w-hi, then C.  Scalar: xh0, xh1 (its
        # later per-group ACTIVATEs queue behind cheap dma dispatches only).
        # GpSimd picks up xh2/xh3 inside the loop (their buffer-reuse waits
        # must not block a queue that has timely work).
        w_sb = [w_pool.tile([P, KC, 4 * P], bf16, name=f"w{hb}") for hb in range(HB)]
        for hb in range(HB):
            for a0, a1 in _WBLOCKS:
                nc.sync.dma_start(w_sb[hb][:, a0:a1, :], w_r[hb][:, a0:a1, :])

        def alloc_xh():
            return xh_pool.tile([P, KC, 512], bf16, name="xhg")

        xh_tiles = {0: alloc_xh(), 1: alloc_xh()}
        for n in (0, 1):
            for a0, a1 in _WBLOCKS:
                nc.scalar.dma_start(
                    xh_tiles[n][:, a0:a1, :], xh_r[n][:, a0:a1, :]
                )

        # C slice: one 2MB DMA behind the w stream (first epilogue needs it
        # at ~40us; it lands ~35us without stealing bandwidth from the
        # critical first-group stream).
        c_all = c_pool.tile([P, HB, B], f32)
        nc.sync.dma_start(c_all[:], c_r[:, :, :])

        for n in range(NT):
            ns = slice(n * 512, (n + 1) * 512)
            # prefetch the n+1 tile one iteration ahead on GpSimd (slot
            # frees when the n-1 groups finish reading their generation)
            if n >= 1 and n + 1 < NT:
                xh_tiles[n + 1] = alloc_xh()
                nc.gpsimd.dma_start(
                    xh_tiles[n + 1][:, 0:16, :], xh_r[n + 1][:, 0:16, :]
                )
                nc.gpsimd.dma_start(
                    xh_tiles[n + 1][:, 16:KC, :], xh_r[n + 1][:, 16:KC, :]
                )
            xh = xh_tiles[n]

            for hb in range(HB):
                final = n == NT - 1 and hb == HB - 1
                ps = [
                    psum_pool.tile([P, 512], f32, name=f"ps{g}") for g in range(4)
                ]
                if final:
                    # gate-major (f,i,g,o): each gate's PSUM closes early so
                    # the epilogue chain after the very last matmul is just
                    # o-sigmoid -> h-mul -> store.
                    for g in (1, 0, 2, 3):
                        for k in range(KC):
                            nc.tensor.matmul(
                                ps[g][:],
                                w_sb[hb][:, k, g * P : (g + 1) * P],
                                xh[:, k, :],
                                start=(k == 0),
                                stop=(k == KC - 1),
                            )
                else:
                    for k in range(KC):
                        for g in range(4):
                            nc.tensor.matmul(
                                ps[g][:],
                                w_sb[hb][:, k, g * P : (g + 1) * P],
                                xh[:, k, :],
                                start=(k == 0),
                                stop=(k == KC - 1),
                            )

                # epilogue: activations (+bias) evict PSUM into the packed
                # output tile, then the cell update.  For the final group
                # f,i,g evict first and the cell-state chain (fc/ig/cn/tanh)
                # completes during the o-gate matmul block, so only
                # o-sigmoid + h-mul remain after the last MM.
                gi = n * HB + hb
                ot = out_pool.tile([P, NQ * 512], f32, name="og")

                def q(name):
                    j = QCOL[name]
                    return ot[:, j * 512 : (j + 1) * 512]

                def gate_act(g, col):
                    m = hb * 4 + g
                    nc.scalar.activation(
                        q(col), ps[g][:], ACT_FN[g], bias=bias_all[:, m : m + 1]
                    )

                order = ((1, "f"), (0, "i"), (2, "g")) if final else (
                    (0, "i"), (1, "f"), (2, "g"), (3, "o"))
                for g, col in order:
                    gate_act(g, col)
                fc = tmp_pool.tile([P, 512], f32, name="fc")
                nc.vector.tensor_mul(fc[:], q("f"), c_all[:, hb, ns])
                ig = tmp_pool.tile([P, 512], f32, name="ig")
                nc.vector.tensor_mul(ig[:], q("i"), q("g"))
                nc.vector.tensor_add(q("cn"), ig[:], fc[:])
                th = tmp_pool.tile([P, 512], f32, name="th")
                nc.scalar.activation(th[:], q("cn"), AF.Tanh)
                if final:
                    gate_act(3, "o")
                nc.vector.tensor_mul(q("h"), q("o"), th[:])

                if final:
                    # f,i,g,cn are done before the o-gate matmul block ends:
                    # store them early on the (idle) Sync queue; the tail
                    # after the last MM is just o-store + h-store on Scalar.
                    nc.sync.dma_start(out_r[gi][:, 0 : 4 * 512], ot[:, 0 : 4 * 512])
                    nc.scalar.dma_start(
                        out_r[gi][:, 4 * 512 : 5 * 512], ot[:, 4 * 512 : 5 * 512]
                    )
                    nc.scalar.dma_start(
                        out_r[gi][:, 5 * 512 : 6 * 512], ot[:, 5 * 512 : 6 * 512]
                    )
                else:
                    nc.sync.dma_start(out_r[gi][:, :], ot[:])

    nc.compile()
    return nc


def _get_program():
    if "nc" not in _CACHE:
        _CACHE["nc"] = _build_program()
    return _CACHE["nc"]


def _gate_row_index(core: int) -> np.ndarray:
    """Global rows of W/b (4H-dim) owned by `core`, in [hb][gate][r] order."""
    idx = []
    for hb in range(HB):
        for g in range(4):
            base = g * H + core * S + hb * P
            idx.extend(range(base, base + P))
    return np.asarray(idx)


def _part_major(a2d: np.ndarray) -> np.ndarray:
    """[K, M] -> [(p a), M] rows ordered p-major (row = p*KC + a)."""
    k, m = a2d.shape
    assert k == P * KC
    return np.ascontiguousarray(
        a2d.reshape(KC, P, m).transpose(1, 0, 2).reshape(k, m)
    )


def kernel(x, h_prev, C_prev, W_ih, b_ih, W_hh, b_hh):
    from concourse.bass_utils import run_bass_kernel_spmd

    nc = _get_program()

    xh_full = np.concatenate([x, h_prev], axis=1).T.astype(_BF16)  # [4096, 2048]
    xh_gens = [
        _part_major(np.ascontiguousarray(xh_full[:, n * 512 : (n + 1) * 512]))
        for n in range(NT)
    ]
    bias_full = (b_ih + b_hh).astype(np.float32)

    in_maps = []
    for c in range(NCORES):
        idx = _gate_row_index(c)
        w_cat = np.concatenate([W_ih[idx], W_hh[idx]], axis=1).astype(_BF16)
        # [1024 rows in [hb][g][r] order, 4096] -> per-hb [4096 k, 512 m]
        w_hb = w_cat.reshape(HB, 4 * P, K)
        cs = C_prev[:, c * S : (c + 1) * S].T  # [256, 2048]
        in_map = {
            "w_lo": _part_major(np.ascontiguousarray(w_hb[0].T)),
            "w_hi": _part_major(np.ascontiguousarray(w_hb[1].T)),
            "bias": np.ascontiguousarray(bias_full[idx].reshape(4 * HB, P).T),
            "c_t": np.ascontiguousarray(
                cs.reshape(HB, P, B).transpose(1, 0, 2).reshape(HB * P, B)
            ),
        }
        for n in range(NT):
            in_map[f"xh{n}"] = xh_gens[n]
        in_maps.append(in_map)

    _CACHE["last_in_maps"] = in_maps
    res = run_bass_kernel_spmd(nc, in_maps, core_ids=list(range(NCORES)))

    # res.results[c]["out"]: [8*128, 3072] -> [n, hb, p, q, col]
    QNAMES = ["f_t", "i_t", "g_t", "cn_t", "o_t", "h_t"]
    full = {}
    parts = [
        res.results[c]["out"].reshape(NT, HB, P, NQ, 512) for c in range(NCORES)
    ]
    for qi, qn in enumerate(QNAMES):
        # rows: core-major h index (c, hb, p); cols: (n, col)
        t = np.concatenate(
            [
                parts[c][:, :, :, qi, :]
                .transpose(1, 2, 0, 3)
                .reshape(S, B)
                for c in range(NCORES)
            ],
            axis=0,
        )  # [H, B]
        full[qn] = np.ascontiguousarray(t.T)

    return (
        full["h_t"],
        full["cn_t"],
        full["f_t"],
        full["i_t"],
        full["g_t"],
        full["o_t"],
    )
